# revision 1
# baseline (speedup 1.0000x reference)
"""Local (windowed, causal) attention on 8 TRN2 NeuronCores.

Shapes (hardcoded): q,k,v [4, 8, 4096, 64] fp32, window=128, look_backward=1.
Sharding: merged batch*heads axis (32) -> 4 heads per core, data parallel.

Device algorithm per head, per key-window c (32 windows of 128 tokens):
  S^T = K_c^T . [Q_c | Q_{c+1}]      (one matmul, contraction over e=64,
                                      out [128 keys, 256 queries] in PSUM;
                                      the two heads of a pair sit in PE row
                                      groups 0-63 / 64-127 and overlap)
  P^T = exp(scale * S^T)             (ACT, PSUM->SBUF)
  P^T[:, :128] *= tri                (GpSimd, causal mask on diagonal block)
  O_w += P^T_block . [V_c | 1]       (two matmuls accumulate the two key-window
                                      contributions per query window; the ones
                                      column accumulates the softmax denominator)
  out_w = O_w[:, :64] * 1/O_w[:, 64] (DVE reciprocal + tensor_scalar_mul)

Host-side marshalling (untimed): Q,K are shipped e-major ([g, 64, 4096]) so the
contraction dim lands in SBUF partitions without on-device transposes; V is
shipped as [g, 32, 128, 65] with a ones column.
"""

import numpy as np

import concourse.bass as bass
import concourse.tile as tile
from concourse import bacc, mybir
from concourse.bass_utils import run_bass_kernel_spmd

B, H, T, E = 4, 8, 4096, 64
WS = 128                      # window size
NW = T // WS                  # 32 windows per sequence
NCORES = 8
GPC = (B * H) // NCORES       # 4 heads per core
SCALE = float(E) ** -0.5
F32 = mybir.dt.float32
F32R = mybir.dt.float32r

# "fp32" (exact), "fp32r_mm1" (fast QK^T), "fp32r_all"
MM_DTYPE = "fp32"


def _qk_dt():
    return F32R if MM_DTYPE in ("fp32r_mm1", "fp32r_all") else F32


def _pv_dt():
    return F32R if MM_DTYPE == "fp32r_all" else F32


def _emit(tc, qT, kT, v, tri, out, repeats=1):
    import contextlib

    nc = tc.nc
    Exp = mybir.ActivationFunctionType.Exp
    mult = mybir.AluOpType.mult

    with contextlib.ExitStack() as ctx:
        qk_pool = ctx.enter_context(tc.tile_pool(name="qk", bufs=2))
        v_pool = ctx.enter_context(tc.tile_pool(name="v", bufs=3))
        o_sb_pool = ctx.enter_context(tc.tile_pool(name="o_sb", bufs=3))
        p_pool = ctx.enter_context(tc.tile_pool(name="p", bufs=4))
        const_pool = ctx.enter_context(tc.tile_pool(name="const", bufs=1))
        s_pool = ctx.enter_context(tc.tile_pool(name="s", bufs=3, space="PSUM"))
        o_ps_pool = ctx.enter_context(tc.tile_pool(name="o_ps", bufs=5, space="PSUM"))
        r_pool = ctx.enter_context(tc.tile_pool(name="r", bufs=6))

        tri_sb = const_pool.tile([WS, WS], F32)
        nc.sync.dma_start(tri_sb[:], tri[:])

        for rep in range(repeats):
            for pair in range(GPC // 2):
                u = f"{rep}_{pair}"
                qT_t = qk_pool.tile([128, T], _qk_dt(), tag="qT", name=f"qT_{u}")
                nc.sync.dma_start(
                    qT_t[:], qT[2 * pair : 2 * pair + 2].rearrange("g e t -> (g e) t")
                )
                kT_t = qk_pool.tile([128, T], _qk_dt(), tag="kT", name=f"kT_{u}")
                nc.sync.dma_start(
                    kT_t[:], kT[2 * pair : 2 * pair + 2].rearrange("g e t -> (g e) t")
                )

                v_t, out_t, ot = [], [], [{}, {}]
                for gg in range(2):
                    g = 2 * pair + gg
                    vt = v_pool.tile([128, NW * 65], _pv_dt(), tag="v", name=f"v_{u}_{gg}")
                    nc.sync.dma_start(
                        vt[:].rearrange("p (w e) -> p w e", e=65),
                        v[g].rearrange("w p e -> p w e"),
                    )
                    v_t.append(vt)
                    outt = o_sb_pool.tile(
                        [128, NW * E], F32, tag="out", name=f"out_{u}_{gg}"
                    )
                    out_t.append(outt)

                for c in range(NW):
                    n = 256 if c < NW - 1 else 128
                    s_t = []
                    # both heads' QK^T back-to-back: disjoint PE row groups overlap
                    for gg in range(2):
                        p0 = 64 * gg
                        st = s_pool.tile([128, 256], F32, tag="s", name=f"s_{u}_{gg}_{c}")
                        nc.tensor.matmul(
                            st[:, :n],
                            lhsT=kT_t[p0 : p0 + 64, WS * c : WS * (c + 1)],
                            rhs=qT_t[p0 : p0 + 64, WS * c : WS * c + n],
                            start=True,
                            stop=True,
                        )
                        s_t.append(st)

                    for gg in range(2):
                        st, vt, outt, od = s_t[gg], v_t[gg], out_t[gg], ot[gg]
                        p_t = p_pool.tile([128, 256], _pv_dt(), tag="p", name=f"p_{u}_{gg}_{c}")
                        nc.scalar.activation(p_t[:, :n], st[:, :n], Exp, scale=SCALE)
                        # causal mask on the diagonal block (keys j valid for i>=j)
                        nc.gpsimd.tensor_tensor(
                            p_t[:, :WS], p_t[:, :WS], tri_sb[:], op=mult
                        )

                        # PV for queries of window c (2nd contribution unless c==0)
                        if c == 0:
                            od[0] = o_ps_pool.tile(
                                [128, 65], F32, tag="o", name=f"o_{u}_{gg}_0"
                            )
                        nc.tensor.matmul(
                            od[c][:],
                            lhsT=p_t[:, :WS],
                            rhs=vt[:, 65 * c : 65 * c + 65],
                            start=(c == 0),
                            stop=True,
                            skip_group_check=True,
                        )
                        # normalize window c -> SBUF out tile
                        rc = r_pool.tile([128, 1], F32, tag="rc", name=f"rc_{u}_{gg}_{c}")
                        nc.vector.reciprocal(rc[:], od[c][:, 64:65])
                        nc.vector.tensor_scalar_mul(
                            outt[:, E * c : E * (c + 1)], od[c][:, 0:E], rc[:]
                        )
                        del od[c]

                        # PV for queries of window c+1 (1st contribution)
                        if c < NW - 1:
                            od[c + 1] = o_ps_pool.tile(
                                [128, 65], F32, tag="o", name=f"o_{u}_{gg}_{c + 1}"
                            )
                            nc.tensor.matmul(
                                od[c + 1][:],
                                lhsT=p_t[:, WS : 2 * WS],
                                rhs=vt[:, 65 * c : 65 * c + 65],
                                start=True,
                                stop=False,
                                skip_group_check=True,
                            )

                for gg in range(2):
                    g = 2 * pair + gg
                    nc.sync.dma_start(
                        out[g].rearrange("(w p) e -> p w e", p=WS),
                        out_t[gg][:].rearrange("p (w e) -> p w e", e=E),
                    )


_CACHE = {}


def _build(repeats=1):
    key = (repeats, MM_DTYPE)
    if key in _CACHE:
        return _CACHE[key]
    nc = bacc.Bacc(
        "TRN2",
        target_bir_lowering=False,
        debug=False,
        num_devices=NCORES,
    )
    qT = nc.dram_tensor("qT", [GPC, E, T], _qk_dt(), kind="ExternalInput").ap()
    kT = nc.dram_tensor("kT", [GPC, E, T], _qk_dt(), kind="ExternalInput").ap()
    v = nc.dram_tensor("v", [GPC, NW, WS, E + 1], _pv_dt(), kind="ExternalInput").ap()
    tri = nc.dram_tensor("tri", [WS, WS], F32, kind="ExternalInput").ap()
    out = nc.dram_tensor("out", [GPC, T, E], F32, kind="ExternalOutput").ap()

    with tile.TileContext(nc) as tc:
        _emit(tc, qT, kT, v, tri, out, repeats=repeats)
    nc.compile()
    _CACHE[key] = nc
    return nc


def _prep_in_maps(q, k, v):
    qm = np.ascontiguousarray(q, dtype=np.float32).reshape(B * H, T, E)
    km = np.ascontiguousarray(k, dtype=np.float32).reshape(B * H, T, E)
    vm = np.ascontiguousarray(v, dtype=np.float32).reshape(B * H, NW, WS, E)
    tri = np.triu(np.ones((WS, WS), dtype=np.float32))  # tri[j, i] = i >= j

    in_maps = []
    for i in range(NCORES):
        sl = slice(GPC * i, GPC * (i + 1))
        qT = np.ascontiguousarray(qm[sl].transpose(0, 2, 1))
        kT = np.ascontiguousarray(km[sl].transpose(0, 2, 1))
        vaug = np.ones((GPC, NW, WS, E + 1), dtype=np.float32)
        vaug[..., :E] = vm[sl]
        in_maps.append({"qT": qT, "kT": kT, "v": vaug, "tri": tri})
    return in_maps


class _Runner:
    """Cached PJRT executor: traces/compiles the NEFF-wrapped jit once and
    reuses it across calls (run_bass_via_pjrt rebuilds it per call, which
    reloads the NEFF on every invocation)."""

    def __init__(self, nc, donate=True):
        import jax
        from jax.experimental.shard_map import shard_map
        from jax.sharding import Mesh, PartitionSpec

        from concourse import bass2jax as b2j

        b2j.install_neuronx_cc_hook()
        self._jax = jax
        self.nc = nc
        part_name = nc.partition_id_tensor.name if nc.partition_id_tensor else None
        in_names, out_names, out_avals, zero_outs = [], [], [], []
        for alloc in nc.m.functions[0].allocations:
            if not isinstance(alloc, mybir.MemoryLocationSet):
                continue
            name = alloc.memorylocations[0].name
            if alloc.kind == "ExternalInput":
                if name != part_name:
                    in_names.append(name)
            elif alloc.kind == "ExternalOutput":
                out_names.append(name)
                shape = tuple(alloc.tensor_shape)
                dtype = mybir.dt.np(alloc.dtype)
                out_avals.append(jax.core.ShapedArray(shape, dtype))
                zero_outs.append(np.zeros(shape, dtype))
        self.in_names, self.out_names = in_names, out_names
        self.out_avals, self.zero_outs = out_avals, zero_outs
        n_params, n_outs = len(in_names), len(out_names)
        all_names = in_names + out_names
        if part_name is not None:
            all_names = all_names + [part_name]

        def _body(*args):
            operands = list(args)
            if part_name is not None:
                operands.append(b2j.partition_id_tensor())
            return tuple(
                b2j._bass_exec_p.bind(
                    *operands,
                    out_avals=tuple(out_avals),
                    in_names=tuple(all_names),
                    out_names=tuple(out_names),
                    lowering_input_output_aliases=(),
                    sim_require_finite=True,
                    sim_require_nnan=True,
                    nc=nc,
                )
            )

        devices = jax.devices()[:NCORES]
        mesh = Mesh(np.asarray(devices), ("core",))
        self.mesh = mesh
        self.in_sharding = jax.sharding.NamedSharding(mesh, PartitionSpec("core"))
        self.jitted = jax.jit(
            shard_map(
                _body,
                mesh=mesh,
                in_specs=(PartitionSpec("core"),) * (n_params + n_outs),
                out_specs=(PartitionSpec("core"),) * n_outs,
                check_rep=False,
            ),
            donate_argnums=(
                tuple(range(n_params, n_params + n_outs)) if donate else ()
            ),
            keep_unused=True,
        )

    def __call__(self, in_maps):
        concat_in = [
            np.concatenate([np.asarray(m[nm]) for m in in_maps], axis=0)
            for nm in self.in_names
        ]
        concat_zeros = [
            np.zeros((NCORES * z.shape[0], *z.shape[1:]), z.dtype)
            for z in self.zero_outs
        ]
        out_arrs = self.jitted(*concat_in, *concat_zeros)
        return [
            {
                nm: np.asarray(out_arrs[i]).reshape(
                    NCORES, *self.out_avals[i].shape
                )[c]
                for i, nm in enumerate(self.out_names)
            }
            for c in range(NCORES)
        ]


    def bench(self, in_maps, ncalls=10):
        """Stage inputs on device once, then time jitted calls. Returns
        (min_seconds, last_outputs_as_core0_np)."""
        import time

        jax = self._jax
        concat_in = [
            jax.device_put(
                np.concatenate([np.asarray(m[nm]) for m in in_maps], axis=0),
                self.in_sharding,
            )
            for nm in self.in_names
        ]
        concat_zeros = [
            jax.device_put(
                np.zeros((NCORES * z.shape[0], *z.shape[1:]), z.dtype),
                self.in_sharding,
            )
            for z in self.zero_outs
        ]
        jax.block_until_ready(concat_in)
        jax.block_until_ready(concat_zeros)
        out = self.jitted(*concat_in, *concat_zeros)  # warm
        jax.block_until_ready(out)
        times = []
        for _ in range(ncalls):
            t0 = time.perf_counter()
            out = self.jitted(*concat_in, *concat_zeros)
            jax.block_until_ready(out)
            times.append(time.perf_counter() - t0)
        return times, out


def _get_runner(repeats=1, donate=True):
    key = ("runner", repeats, MM_DTYPE, donate)
    if key not in _CACHE:
        _CACHE[key] = _Runner(_build(repeats=repeats), donate=donate)
    return _CACHE[key]


def run(q, k, v, repeats=1, **kw):
    runner = _get_runner(repeats=repeats)
    in_maps = _prep_in_maps(q, k, v)
    results = runner(in_maps)
    outs = [results[i]["out"] for i in range(NCORES)]
    full = np.concatenate(outs, axis=0).reshape(B, H, T, E)
    return full, None


def kernel(q, k, v):
    full, _ = run(q, k, v)
    return full

